# revision 10
# baseline (speedup 1.0000x reference)
"""Trainium2 Bass kernel for masked bi-linear attention.

Computes, for full inputs
    k:    [B, KL, E] f32
    q:    [B, Q,  E] f32
    W:    [E, E]     f32
    mask: [B, Q, KL] i32 (0/1)
the reference
    qw    = q @ W                      [B, Q, E]
    s     = qw @ k^T                   [B, Q, KL]
    p     = softmax(s, axis=-1) * mask
    out   = p @ k                      [B, Q, E]

Sharding: data-parallel over B across 8 NeuronCores (2 batches/core),
W replicated. Each core runs the same Bass program on its B-slice.

Precision: q/W/k in fp16 for the qw and score matmuls (score abs error
must stay << 1 since softmax amplifies it); softmax in fp32 on ACT/DVE.

The PV matmul (p @ k) runs in fp8 e4m3 with DoubleRow perf mode: the
stationary holds two p^T chunks (2 fp8 weights/cell) and the moving
operand streams the two matching k8 chunks, halving the PE cycles of
the PV phase.  fp8 quantization of k alone would cost ~2.7e-2 rel
error (over the 2e-2 gate) because softmax rows are near one-hot, so a
top-1 sparse correction restores it: per query row, the largest masked
probability e_top and its key index are found with DVE max/max_index,
the fp8 residual row dk8[idx] = fp8(k - fp8(k)) is gathered from a
DRAM scratch by indirect DMA, and one extra fp8 matmul with stationary
diag(e_top) accumulates e_top * dk8[idx] into the same PSUM group.
Measured rel l2 err ~5e-3 (vs 3e-3 for the all-fp16 PV).

Engine layout (per q-tile): PE does only matmuls (scores, qw share,
deferred PV); all transposes (q^T, k^T, p^T) run on the DMA xbar
(dma_start_transpose, 16-bit, all on the sync ring -- concurrent
transposes on both HWDGE rings corrupt); the score psum is consumed
directly by the DVE row-max and the ACT exp (no psum->SBUF copy); the
fp16/fp8 input casts sit on ACT (q16/k16) and DVE (k8/dk8); max8 and
find_index8 for the top-1 correction on DVE; mask loads, dk8 spills,
residual gathers and output stores on the gpsimd (SWDGE) ring.
"""

import numpy as np

import concourse.bacc as bacc
import concourse.bass as bass
import concourse.mybir as mybir
import concourse.tile as tile
from concourse.bass_utils import run_bass_kernel_spmd
from concourse.masks import make_identity
from contextlib import ExitStack

dt = mybir.dt
AF = mybir.ActivationFunctionType
ALU = mybir.AluOpType
AX = mybir.AxisListType
PM = mybir.MatmulPerfMode

P = 128

N_CORES = 8
B, Q_LEN, K_LEN, EMB = 16, 2048, 2048, 1024


def emit_attention(ctx, tc, k_ap, q_ap, w_ap, mask_ap, out_ap, dk8_ap,
                   Bl, Q, KL, E, QB=512):
    """Emit the per-core attention program.

    k_ap [Bl, KL, E], q_ap [Bl, Q, E], w_ap [E, E], mask_ap [Bl, Q, KL],
    out_ap [Bl, Q, E], dk8_ap [Bl*KL, E] fp8 scratch (Internal DRAM).
    """
    nc = tc.nc
    f32, bf16, f16, i32 = dt.float32, dt.bfloat16, dt.float16, dt.int32
    f8, u32 = dt.float8e4, dt.uint32

    assert Q % QB == 0 and QB % P == 0 and KL % P == 0 and E % P == 0
    EC = E // P          # e (contraction for qw) chunks
    KC = KL // P         # k chunks
    FC = E // P          # f chunks (qw output tiles)
    nqb = Q // QB
    qt_per_b = QB // P
    KB = min(512, KL)    # score psum block (<= 1 bank)
    nkb = KL // KB
    EB = min(512, E)     # PV psum block
    neb = E // EB

    const = ctx.enter_context(tc.tile_pool(name="const", bufs=1))
    ident = const.tile([P, P], f32)
    make_identity(nc, ident[:])
    id8_t = const.tile([P, P], f8)
    nc.gpsimd.tensor_copy(id8_t[:], ident[:])   # 0/1 exact in fp8
    id8 = id8_t[:]

    big = ctx.enter_context(tc.tile_pool(name="big", bufs=1))
    qio = ctx.enter_context(tc.tile_pool(name="qio", bufs=5))
    q16p = ctx.enter_context(tc.tile_pool(name="q16p", bufs=3))
    mio = ctx.enter_context(tc.tile_pool(name="mio", bufs=2))
    ptp = ctx.enter_context(tc.tile_pool(name="ptp", bufs=4))
    work = ctx.enter_context(tc.tile_pool(name="work", bufs=2))
    small = ctx.enter_context(tc.tile_pool(name="small", bufs=3))
    psum = ctx.enter_context(tc.tile_pool(name="psum", bufs=6, space="PSUM"))
    psum_o = ctx.enter_context(tc.tile_pool(name="psum_o", bufs=1, space="PSUM"))

    # ---- W: loaded once per core as fp16; the DMA+cast emission happens
    # after the first q-block's loads so the kernel head starts on the q
    # pipeline instead of waiting for W
    wH = big.tile([P, EC * E], f16, tag="wH")

    def emit_w_load():
        for ec in range(EC):
            win = qio.tile([P, E], f32, tag="qin", name="win")
            nc.sync.dma_start(win[:], w_ap[ec * P:(ec + 1) * P, :])
            nc.scalar.copy(wH[:, ec * E:(ec + 1) * E], win[:])

    # deferred-PV state: [b, row0, spb, rz, k8t, e8q, idxa, (pT8, dkg)]
    pending = []

    def pv_prep(st):
        # p^T via the DMA xbar (SBUF->SBUF, bf16) on the sync queue,
        # then the fp8 cast for the DoubleRow stationary (ACT), and the
        # residual-row gather for the top-1 correction (gpsimd)
        spb, idxa = st[2], st[6]
        pT = ptp.tile([P, KC, P], bf16, tag="pT", name="pT", bufs=2)
        nc.sync.dma_start_transpose(pT[:], spb[:])
        pT8 = ptp.tile([P, KC, P], f8, tag="pT8", name="pT8", bufs=2)
        nc.scalar.copy(pT8[:], pT[:])
        dkg = mio.tile([P, E], f8, tag="dkg", name="dkg", bufs=2)
        nc.gpsimd.indirect_dma_start(
            out=dkg[:], out_offset=None,
            in_=dk8_ap[:],
            in_offset=bass.IndirectOffsetOnAxis(ap=idxa[:, :1], axis=0))
        st.append((pT8, dkg))
        return st[-1]

    def pv_mms(st, prep):
        b, row0, spb, rz, k8t, e8q = st[:6]
        pT8, dkg = prep
        # per-row correction scale diag(e_top) in fp8
        dg8 = small.tile([P, P], f8, tag="dg8", name="dg8", bufs=2)
        nc.vector.tensor_scalar_mul(dg8[:], id8, e8q[:, 0:1])
        po = [psum_o.tile([P, EB], f32, tag=f"po{eh}", name=f"po{eh}")
              for eh in range(neb)]
        for c2 in range(KC // 2):
            for eh in range(neb):
                nc.tensor.matmul(
                    po[eh][:], pT8[:, 2 * c2:2 * c2 + 2, :],
                    k8t[:, 2 * c2:2 * c2 + 2, eh * EB:(eh + 1) * EB],
                    start=(c2 == 0), stop=False,
                    perf_mode=PM.DoubleRow)
        for eh in range(neb):
            nc.tensor.matmul(
                po[eh][:], dg8[:], dkg[:, eh * EB:(eh + 1) * EB],
                start=False, stop=True)
        for eh in range(neb):
            ot = mio.tile([P, EB], f32, tag="ot", name="ot")
            nc.scalar.activation(ot[:], po[eh][:], AF.Copy, scale=rz[:])
            nc.gpsimd.dma_start(
                out_ap[b, row0: row0 + P, eh * EB:(eh + 1) * EB], ot[:])

    # ---- q loads: DMAs may be issued ahead (prefetched) of the cast
    # emission so they don't queue behind mask DMAs
    def emit_qin_dmas(b, qb, qts):
        tiles = []
        for qt in qts:
            qin = qio.tile([P, E], f32, tag="qin", name="qin")
            nc.sync.dma_start(
                qin[:], q_ap[b, qb * QB + qt * P: qb * QB + (qt + 1) * P, :])
            tiles.append(qin)
        return tiles

    def emit_q16(qins):
        outs = []
        for qin in qins:
            q16 = q16p.tile([P, E], f16, tag="q16", name="q16", bufs=5)
            nc.scalar.copy(q16[:], qin[:])
            outs.append(q16)
        return outs

    def emit_block_qT(b, qb, pre16):
        qT = big.tile([P, EC, QB], f16, tag="qTh", name="qT")
        q16s = pre16 + emit_q16(
            emit_qin_dmas(b, qb, range(len(pre16), qt_per_b)))
        for qt in range(qt_per_b):
            nc.sync.dma_start_transpose(
                qT[:, :, qt * P:(qt + 1) * P], q16s[qt][:])
        return qT

    def emit_block_qw(qT):
        qwT = big.tile([P, FC * QB], f16, tag="qwTh", name="qwT")
        for fc in range(FC):
            ps = psum.tile([P, QB], f32, tag="ps", name="ps")
            for ec in range(EC):
                nc.tensor.matmul(
                    ps[:], wH[:, ec * E + fc * P: ec * E + (fc + 1) * P],
                    qT[:, ec, :], start=(ec == 0), stop=(ec == EC - 1))
            nc.scalar.copy(qwT[:, fc * QB:(fc + 1) * QB], ps[:])
        return qwT

    def emit_k_phase(b):
        k8 = big.tile([P, KC, E], f8, tag="k8", name="k8", bufs=2)
        kTh = big.tile([P, EC, KL], f16, tag="kTh", name="kTh")

        k16_prev = [None]

        def chunk(kc):
            kin = qio.tile([P, E], f32, tag="qin", name="kin")
            nc.sync.dma_start(kin[:], k_ap[b, kc * P:(kc + 1) * P, :])
            # the previous chunk's kTh xbar transpose goes on the ring
            # AFTER the next kin issue so the DMA stream never stalls
            # behind the transpose ucode's wait on the cast
            if k16_prev[0] is not None:
                pc, pk16 = k16_prev[0]
                nc.sync.dma_start_transpose(
                    kTh[:, :, pc * P:(pc + 1) * P], pk16[:])
            # fp8 rounding copy for the PV matmul rhs + residual row
            nc.vector.tensor_copy(k8[:, kc, :], kin[:])
            dk8 = q16p.tile([P, E], f8, tag="dk8", name="dk8", bufs=3)
            nc.vector.tensor_tensor(out=dk8[:], in0=kin[:],
                                    in1=k8[:, kc, :], op=ALU.subtract)
            nc.gpsimd.dma_start(
                dk8_ap[b * KL + kc * P: b * KL + (kc + 1) * P, :], dk8[:])
            k16 = q16p.tile([P, E], f16, tag="q16", name="k16", bufs=5)
            nc.scalar.copy(k16[:], kin[:])
            k16_prev[0] = (kc, k16)

        for kc in range(KC):
            chunk(kc)
        pc, pk16 = k16_prev[0]
        nc.sync.dma_start_transpose(kTh[:, :, pc * P:(pc + 1) * P], pk16[:])
        return k8, kTh

    for b in range(Bl):
        # first q-block prep runs before the K phase: its qw matmuls keep
        # the PE busy while the k DMA stream lands; k8 is
        # double-buffered so the previous batch's deferred PV drains
        # during the k load
        qT = emit_block_qT(b, 0, [])

        if b == 0:
            emit_w_load()
        qwT = emit_block_qw(qT)
        k8, kTh = emit_k_phase(b)

        qin_pre, q16_pre = [], []
        for qb in range(nqb):
            if qb > 0:
                qT = emit_block_qT(b, qb, q16_pre)
                q16_pre = []
                qwT = emit_block_qw(qT)

            for qt in range(qt_per_b):
                row0 = qb * QB + qt * P
                # mask prefetch on the gpsimd (SWDGE) ring, one [P, KB]
                # tile per score block, consumed after this tile's exp
                mts = []
                for kb in range(nkb):
                    mt = mio.tile([P, KB], i32, tag="mask", name="mt",
                                  bufs=6)
                    nc.gpsimd.dma_start(
                        mt[:], mask_ap[b, row0: row0 + P,
                                       kb * KB:(kb + 1) * KB])
                    mts.append(mt)

                # xbar-transpose + fp8-cast + residual-gather for the
                # newest deferred tile: its mask multiplies have
                # finished by the time the sync queue reaches this
                if pending:
                    pv_prep(pending[-1])

                # scores stay in PSUM: the DVE row-max and the ACT exp
                # read the banks directly (no psum->SBUF copy)
                pss = []
                mx = small.tile([P, nkb], f32, tag="mx", name="mx")
                for kb in range(nkb):
                    ps_s = psum.tile([P, KB], f32, tag="ps", name="ps_s")
                    for fc in range(FC):
                        nc.tensor.matmul(
                            ps_s[:],
                            qwT[:, fc * QB + qt * P: fc * QB + (qt + 1) * P],
                            kTh[:, fc, kb * KB:(kb + 1) * KB],
                            start=(fc == 0), stop=(fc == FC - 1))
                    nc.vector.tensor_reduce(
                        mx[:, kb:kb + 1], ps_s[:],
                        axis=AX.X, op=ALU.max)
                    pss.append(ps_s)

                negm = small.tile([P, 1], f32, tag="negm", name="negm")
                nc.vector.tensor_reduce(negm[:], mx[:], axis=AX.X,
                                        op=ALU.max, negate=True)

                spb = work.tile([P, KL], bf16, tag="spb", name="spb")
                zs = small.tile([P, nkb], f32, tag="zs", name="zs")
                for kb in range(nkb):
                    blk = slice(kb * KB, (kb + 1) * KB)
                    nc.scalar.activation(spb[:, blk], pss[kb][:], AF.Exp,
                                         bias=negm[:],
                                         accum_out=zs[:, kb:kb + 1])
                    nc.vector.scalar_tensor_tensor(
                        out=spb[:, blk], in0=mts[kb][:], scalar=1.0,
                        in1=spb[:, blk], op0=ALU.mult, op1=ALU.mult)
                z = small.tile([P, 1], f32, tag="z", name="z")
                nc.vector.tensor_reduce(z[:], zs[:], axis=AX.X, op=ALU.add)
                rz = small.tile([P, 1], f32, tag="rz", name="rz")
                nc.vector.reciprocal(rz[:], z[:])

                # top-1 masked probability (value + key index) for the
                # fp8 residual correction
                m8 = small.tile([P, 8], bf16, tag="m8", name="m8")
                mi = small.tile([P, 8], u32, tag="mi", name="mi")
                nc.vector.max(m8[:], spb[:])
                nc.vector.max_index(mi[:], m8[:], spb[:])
                idxa = small.tile([P, 1], u32, tag="idxa", name="idxa")
                nc.vector.tensor_scalar_add(idxa[:], mi[:, 0:1], b * KL)
                # f32 copy of the top value; the write into the fp8
                # diag tile rounds it to the same e4m3 value the
                # DoubleRow stationary used
                e8q = small.tile([P, 1], f32, tag="e8q", name="e8q")
                nc.vector.tensor_copy(e8q[:], m8[:, 0:1])

                # PV deferred by two tiles so the xbar transpose is
                # never on the critical path
                if len(pending) > 1:
                    st = pending.pop(0)
                    pv_mms(st, st[7])
                pending.append([b, row0, spb, rz, k8, e8q, idxa])

                # prefetch the next block's q rows (DMA a tile before
                # the fp16 cast, so neither the sync queue nor ACT
                # stalls): the block head then has no input dependency
                if qt == qt_per_b - 3 and qb + 1 < nqb:
                    qin_pre = emit_qin_dmas(b, qb + 1, range(4))
                if qt == qt_per_b - 2 and qin_pre:
                    q16_pre = emit_q16(qin_pre)
                    qin_pre = []

    for st in pending:
        if len(st) <= 7:
            pv_prep(st)
    while pending:
        st = pending.pop(0)
        pv_mms(st, st[7])


def build_program(Bl, Q, KL, E, QB=512):
    nc = bacc.Bacc("TRN2", target_bir_lowering=False, debug=False)
    k_t = nc.dram_tensor("k", [Bl, KL, E], dt.float32, kind="ExternalInput")
    q_t = nc.dram_tensor("q", [Bl, Q, E], dt.float32, kind="ExternalInput")
    w_t = nc.dram_tensor("W", [E, E], dt.float32, kind="ExternalInput")
    m_t = nc.dram_tensor("mask", [Bl, Q, KL], dt.int32, kind="ExternalInput")
    o_t = nc.dram_tensor("out", [Bl, Q, E], dt.float32, kind="ExternalOutput")
    dk8_t = nc.dram_tensor("dk8s", [Bl * KL, E], dt.float8e4, kind="Internal")
    with tile.TileContext(nc) as tc:
        with ExitStack() as ctx:
            emit_attention(ctx, tc, k_t.ap(), q_t.ap(), w_t.ap(), m_t.ap(),
                           o_t.ap(), dk8_t.ap(), Bl, Q, KL, E, QB=QB)
    nc.compile()
    return nc


def kernel(k: np.ndarray, q: np.ndarray, W: np.ndarray, mask: np.ndarray,
           **run_kwargs) -> np.ndarray:
    assert k.shape == (B, K_LEN, EMB) and q.shape == (B, Q_LEN, EMB)
    assert W.shape == (EMB, EMB) and mask.shape == (B, Q_LEN, K_LEN)
    Bl = B // N_CORES
    nc = build_program(Bl, Q_LEN, K_LEN, EMB)
    in_maps = []
    for c in range(N_CORES):
        sl = slice(c * Bl, (c + 1) * Bl)
        in_maps.append({
            "k": np.ascontiguousarray(k[sl], dtype=np.float32),
            "q": np.ascontiguousarray(q[sl], dtype=np.float32),
            "W": np.ascontiguousarray(W, dtype=np.float32),
            "mask": np.ascontiguousarray(mask[sl], dtype=np.int32),
        })
    res = run_bass_kernel_spmd(nc, in_maps, core_ids=list(range(N_CORES)),
                               **run_kwargs)
    out = np.concatenate([r["out"] for r in res.results], axis=0)
    if run_kwargs.get("trace"):
        kernel.last_exec_time_ns = res.exec_time_ns
        kernel.last_result = res
    return out


kernel.last_exec_time_ns = None
kernel.last_result = None


# revision 11
# speedup vs baseline: 1.0274x; 1.0274x over previous
"""Trainium2 Bass kernel for masked bi-linear attention.

Computes, for full inputs
    k:    [B, KL, E] f32
    q:    [B, Q,  E] f32
    W:    [E, E]     f32
    mask: [B, Q, KL] i32 (0/1)
the reference
    qw    = q @ W                      [B, Q, E]
    s     = qw @ k^T                   [B, Q, KL]
    p     = softmax(s, axis=-1) * mask
    out   = p @ k                      [B, Q, E]

Sharding: data-parallel over B across 8 NeuronCores (2 batches/core),
W replicated. Each core runs the same Bass program on its B-slice.

Precision: q/W/k in fp16 for the qw and score matmuls (score abs error
must stay << 1 since softmax amplifies it); softmax in fp32 on ACT/DVE.

The PV matmul (p @ k) runs in fp8 e4m3 with DoubleRow perf mode: the
stationary holds two p^T chunks (2 fp8 weights/cell) and the moving
operand streams the two matching k8 chunks, halving the PE cycles of
the PV phase.  fp8 quantization of k alone would cost ~2.7e-2 rel
error (over the 2e-2 gate) because softmax rows are near one-hot, so a
top-1 sparse correction restores it: per query row, the largest masked
probability e_top and its key index are found with DVE max/max_index,
the fp8 residual row dk8[idx] = fp8(k - fp8(k)) is gathered from a
DRAM scratch by indirect DMA, and one extra fp8 matmul with stationary
diag(e_top) accumulates e_top * dk8[idx] into the same PSUM group.
Measured rel l2 err ~5e-3 (vs 3e-3 for the all-fp16 PV).

Engine layout (per q-tile): PE does only matmuls (scores, qw share,
deferred PV); all transposes (q^T, k^T, p^T) run on the DMA xbar
(dma_start_transpose, 16-bit, all on the sync ring -- concurrent
transposes on both HWDGE rings corrupt); the score psum is consumed
directly by the DVE row-max and the ACT exp (no psum->SBUF copy); the
fp16/fp8 input casts sit on ACT (q16/k16) and DVE (k8/dk8); max8 and
find_index8 for the top-1 correction on DVE; mask loads, dk8 spills,
residual gathers and output stores on the gpsimd (SWDGE) ring.
"""

import numpy as np

import concourse.bacc as bacc
import concourse.bass as bass
import concourse.mybir as mybir
import concourse.tile as tile
from concourse.bass_utils import run_bass_kernel_spmd
from concourse.masks import make_identity
from contextlib import ExitStack

dt = mybir.dt
AF = mybir.ActivationFunctionType
ALU = mybir.AluOpType
AX = mybir.AxisListType
PM = mybir.MatmulPerfMode

P = 128

N_CORES = 8
B, Q_LEN, K_LEN, EMB = 16, 2048, 2048, 1024


def emit_attention(ctx, tc, k_ap, q_ap, w_ap, mask_ap, out_ap, dk8_ap,
                   Bl, Q, KL, E, QB=512):
    """Emit the per-core attention program.

    k_ap [Bl, KL, E], q_ap [Bl, Q, E], w_ap [E, E], mask_ap [Bl, Q, KL],
    out_ap [Bl, Q, E], dk8_ap [Bl*KL, E] fp8 scratch (Internal DRAM).
    """
    nc = tc.nc
    f32, bf16, f16, i32 = dt.float32, dt.bfloat16, dt.float16, dt.int32
    f8, u32 = dt.float8e4, dt.uint32

    assert Q % QB == 0 and QB % P == 0 and KL % P == 0 and E % P == 0
    EC = E // P          # e (contraction for qw) chunks
    KC = KL // P         # k chunks
    FC = E // P          # f chunks (qw output tiles)
    nqb = Q // QB
    qt_per_b = QB // P
    KB = min(512, KL)    # score psum block (<= 1 bank)
    nkb = KL // KB
    EB = min(512, E)     # PV psum block
    neb = E // EB

    const = ctx.enter_context(tc.tile_pool(name="const", bufs=1))
    ident = const.tile([P, P], f32)
    make_identity(nc, ident[:])
    id8_t = const.tile([P, P], f8)
    nc.gpsimd.tensor_copy(id8_t[:], ident[:])   # 0/1 exact in fp8
    id8 = id8_t[:]

    big = ctx.enter_context(tc.tile_pool(name="big", bufs=1))
    qio = ctx.enter_context(tc.tile_pool(name="qio", bufs=5))
    q16p = ctx.enter_context(tc.tile_pool(name="q16p", bufs=3))
    mio = ctx.enter_context(tc.tile_pool(name="mio", bufs=2))
    ptp = ctx.enter_context(tc.tile_pool(name="ptp", bufs=4))
    work = ctx.enter_context(tc.tile_pool(name="work", bufs=2))
    small = ctx.enter_context(tc.tile_pool(name="small", bufs=3))
    psum = ctx.enter_context(tc.tile_pool(name="psum", bufs=6, space="PSUM"))
    psum_o = ctx.enter_context(tc.tile_pool(name="psum_o", bufs=1, space="PSUM"))

    # ---- W: loaded once per core as fp16; the DMA+cast emission happens
    # after the first q-block's loads so the kernel head starts on the q
    # pipeline instead of waiting for W
    wH = big.tile([P, EC * E], f16, tag="wH")

    def emit_w_load():
        for ec in range(EC):
            win = qio.tile([P, E], f32, tag="qin", name="win")
            nc.sync.dma_start(win[:], w_ap[ec * P:(ec + 1) * P, :])
            nc.scalar.copy(wH[:, ec * E:(ec + 1) * E], win[:])

    # deferred-PV state: [b, row0, spb, rz, k8t, e8q, idxa, (pT8, dkg)]
    pending = []

    def pv_prep(st):
        # p^T via the DMA xbar (SBUF->SBUF, bf16) on the sync queue,
        # then the fp8 cast for the DoubleRow stationary (ACT), and the
        # residual-row gather for the top-1 correction (gpsimd)
        spb, idxa = st[2], st[6]
        pT = ptp.tile([P, KC, P], bf16, tag="pT", name="pT", bufs=2)
        nc.sync.dma_start_transpose(pT[:], spb[:])
        pT8 = ptp.tile([P, KC, P], f8, tag="pT8", name="pT8", bufs=2)
        nc.vector.tensor_copy(pT8[:], pT[:])
        dkg = mio.tile([P, E], f8, tag="dkg", name="dkg", bufs=2)
        nc.gpsimd.indirect_dma_start(
            out=dkg[:], out_offset=None,
            in_=dk8_ap[:],
            in_offset=bass.IndirectOffsetOnAxis(ap=idxa[:, :1], axis=0))
        st.append((pT8, dkg))
        return st[-1]

    def pv_mms(st, prep):
        b, row0, spb, rz, k8t, e8q = st[:6]
        pT8, dkg = prep
        # per-row correction scale diag(e_top) in fp8
        dg8 = small.tile([P, P], f8, tag="dg8", name="dg8", bufs=2)
        nc.vector.tensor_scalar_mul(dg8[:], id8, e8q[:, 0:1])
        po = [psum_o.tile([P, EB], f32, tag=f"po{eh}", name=f"po{eh}")
              for eh in range(neb)]
        for c2 in range(KC // 2):
            for eh in range(neb):
                nc.tensor.matmul(
                    po[eh][:], pT8[:, 2 * c2:2 * c2 + 2, :],
                    k8t[:, 2 * c2:2 * c2 + 2, eh * EB:(eh + 1) * EB],
                    start=(c2 == 0), stop=False,
                    perf_mode=PM.DoubleRow)
        for eh in range(neb):
            nc.tensor.matmul(
                po[eh][:], dg8[:], dkg[:, eh * EB:(eh + 1) * EB],
                start=False, stop=True)
        for eh in range(neb):
            ot = mio.tile([P, EB], f32, tag="ot", name="ot")
            nc.scalar.activation(ot[:], po[eh][:], AF.Copy, scale=rz[:])
            nc.gpsimd.dma_start(
                out_ap[b, row0: row0 + P, eh * EB:(eh + 1) * EB], ot[:])

    # ---- q loads: DMAs may be issued ahead (prefetched) of the cast
    # emission so they don't queue behind mask DMAs
    def emit_qin_dmas(b, qb, qts):
        tiles = []
        for qt in qts:
            qin = qio.tile([P, E], f32, tag="qin", name="qin")
            nc.sync.dma_start(
                qin[:], q_ap[b, qb * QB + qt * P: qb * QB + (qt + 1) * P, :])
            tiles.append(qin)
        return tiles

    def emit_q16(qins):
        outs = []
        for qin in qins:
            q16 = q16p.tile([P, E], f16, tag="q16", name="q16", bufs=5)
            nc.scalar.copy(q16[:], qin[:])
            outs.append(q16)
        return outs

    def emit_block_qT(b, qb, pre16):
        qT = big.tile([P, EC, QB], f16, tag="qTh", name="qT")
        q16s = pre16 + emit_q16(
            emit_qin_dmas(b, qb, range(len(pre16), qt_per_b)))
        for qt in range(qt_per_b):
            nc.sync.dma_start_transpose(
                qT[:, :, qt * P:(qt + 1) * P], q16s[qt][:])
        return qT

    def emit_block_qw(qT):
        qwT = big.tile([P, FC * QB], f16, tag="qwTh", name="qwT")
        for fc in range(FC):
            ps = psum.tile([P, QB], f32, tag="ps", name="ps")
            for ec in range(EC):
                nc.tensor.matmul(
                    ps[:], wH[:, ec * E + fc * P: ec * E + (fc + 1) * P],
                    qT[:, ec, :], start=(ec == 0), stop=(ec == EC - 1))
            nc.scalar.copy(qwT[:, fc * QB:(fc + 1) * QB], ps[:])
        return qwT

    def emit_k_phase(b):
        k8 = big.tile([P, KC, E], f8, tag="k8", name="k8", bufs=2)
        kTh = big.tile([P, EC, KL], f16, tag="kTh", name="kTh")

        k16_prev = [None]

        def chunk(kc):
            kin = qio.tile([P, E], f32, tag="qin", name="kin")
            nc.sync.dma_start(kin[:], k_ap[b, kc * P:(kc + 1) * P, :])
            # the previous chunk's kTh xbar transpose goes on the ring
            # AFTER the next kin issue so the DMA stream never stalls
            # behind the transpose ucode's wait on the cast
            if k16_prev[0] is not None:
                pc, pk16 = k16_prev[0]
                nc.sync.dma_start_transpose(
                    kTh[:, :, pc * P:(pc + 1) * P], pk16[:])
            # fp8 rounding copy for the PV matmul rhs + residual row
            nc.vector.tensor_copy(k8[:, kc, :], kin[:])
            dk8 = q16p.tile([P, E], f8, tag="dk8", name="dk8", bufs=3)
            nc.vector.tensor_tensor(out=dk8[:], in0=kin[:],
                                    in1=k8[:, kc, :], op=ALU.subtract)
            nc.gpsimd.dma_start(
                dk8_ap[b * KL + kc * P: b * KL + (kc + 1) * P, :], dk8[:])
            k16 = q16p.tile([P, E], f16, tag="q16", name="k16", bufs=5)
            nc.scalar.copy(k16[:], kin[:])
            k16_prev[0] = (kc, k16)

        for kc in range(KC):
            chunk(kc)
        pc, pk16 = k16_prev[0]
        nc.sync.dma_start_transpose(kTh[:, :, pc * P:(pc + 1) * P], pk16[:])
        return k8, kTh

    for b in range(Bl):
        # first q-block prep runs before the K phase: its qw matmuls keep
        # the PE busy while the k DMA stream lands; k8 is
        # double-buffered so the previous batch's deferred PV drains
        # during the k load
        qT = emit_block_qT(b, 0, [])

        if b == 0:
            emit_w_load()
        qwT = emit_block_qw(qT)
        k8, kTh = emit_k_phase(b)

        qin_pre, q16_pre = [], []
        for qb in range(nqb):
            if qb > 0:
                qT = emit_block_qT(b, qb, q16_pre)
                q16_pre = []
                qwT = emit_block_qw(qT)

            for qt in range(qt_per_b):
                row0 = qb * QB + qt * P
                # mask prefetch on the gpsimd (SWDGE) ring, one [P, KB]
                # tile per score block, consumed after this tile's exp
                mts = []
                for kb in range(nkb):
                    mt = mio.tile([P, KB], i32, tag="mask", name="mt",
                                  bufs=6)
                    nc.gpsimd.dma_start(
                        mt[:], mask_ap[b, row0: row0 + P,
                                       kb * KB:(kb + 1) * KB])
                    mts.append(mt)

                # xbar-transpose + fp8-cast + residual-gather for the
                # newest deferred tile: its mask multiplies have
                # finished by the time the sync queue reaches this
                if pending:
                    pv_prep(pending[-1])

                # scores stay in PSUM: the DVE row-max and the ACT exp
                # read the banks directly (no psum->SBUF copy)
                pss = []
                mx = small.tile([P, nkb], f32, tag="mx", name="mx")
                for kb in range(nkb):
                    ps_s = psum.tile([P, KB], f32, tag="ps", name="ps_s")
                    for fc in range(FC):
                        nc.tensor.matmul(
                            ps_s[:],
                            qwT[:, fc * QB + qt * P: fc * QB + (qt + 1) * P],
                            kTh[:, fc, kb * KB:(kb + 1) * KB],
                            start=(fc == 0), stop=(fc == FC - 1))
                    nc.vector.tensor_reduce(
                        mx[:, kb:kb + 1], ps_s[:],
                        axis=AX.X, op=ALU.max)
                    pss.append(ps_s)

                negm = small.tile([P, 1], f32, tag="negm", name="negm")
                nc.vector.tensor_reduce(negm[:], mx[:], axis=AX.X,
                                        op=ALU.max, negate=True)

                spb = work.tile([P, KL], bf16, tag="spb", name="spb")
                zs = small.tile([P, nkb], f32, tag="zs", name="zs")
                for kb in range(nkb):
                    blk = slice(kb * KB, (kb + 1) * KB)
                    nc.scalar.activation(spb[:, blk], pss[kb][:], AF.Exp,
                                         bias=negm[:],
                                         accum_out=zs[:, kb:kb + 1])
                    nc.vector.scalar_tensor_tensor(
                        out=spb[:, blk], in0=mts[kb][:], scalar=1.0,
                        in1=spb[:, blk], op0=ALU.mult, op1=ALU.mult)
                z = small.tile([P, 1], f32, tag="z", name="z")
                nc.vector.tensor_reduce(z[:], zs[:], axis=AX.X, op=ALU.add)
                rz = small.tile([P, 1], f32, tag="rz", name="rz")
                nc.vector.reciprocal(rz[:], z[:])

                # top-1 masked probability (value + key index) for the
                # fp8 residual correction
                m8 = small.tile([P, 8], bf16, tag="m8", name="m8")
                mi = small.tile([P, 8], u32, tag="mi", name="mi")
                nc.vector.max(m8[:], spb[:])
                nc.vector.max_index(mi[:], m8[:], spb[:])
                idxa = small.tile([P, 1], u32, tag="idxa", name="idxa")
                nc.vector.tensor_scalar_add(idxa[:], mi[:, 0:1], b * KL)
                # f32 copy of the top value; the write into the fp8
                # diag tile rounds it to the same e4m3 value the
                # DoubleRow stationary used
                e8q = small.tile([P, 1], f32, tag="e8q", name="e8q")
                nc.vector.tensor_copy(e8q[:], m8[:, 0:1])

                # PV deferred by two tiles so the xbar transpose is
                # never on the critical path
                if len(pending) > 1:
                    st = pending.pop(0)
                    pv_mms(st, st[7])
                pending.append([b, row0, spb, rz, k8, e8q, idxa])

                # prefetch the next block's q rows (DMA a tile before
                # the fp16 cast, so neither the sync queue nor ACT
                # stalls): the block head then has no input dependency
                if qt == qt_per_b - 3 and qb + 1 < nqb:
                    qin_pre = emit_qin_dmas(b, qb + 1, range(4))
                if qt == qt_per_b - 2 and qin_pre:
                    q16_pre = emit_q16(qin_pre)
                    qin_pre = []

    for st in pending:
        if len(st) <= 7:
            pv_prep(st)
    while pending:
        st = pending.pop(0)
        pv_mms(st, st[7])


def build_program(Bl, Q, KL, E, QB=512):
    nc = bacc.Bacc("TRN2", target_bir_lowering=False, debug=False)
    k_t = nc.dram_tensor("k", [Bl, KL, E], dt.float32, kind="ExternalInput")
    q_t = nc.dram_tensor("q", [Bl, Q, E], dt.float32, kind="ExternalInput")
    w_t = nc.dram_tensor("W", [E, E], dt.float32, kind="ExternalInput")
    m_t = nc.dram_tensor("mask", [Bl, Q, KL], dt.int32, kind="ExternalInput")
    o_t = nc.dram_tensor("out", [Bl, Q, E], dt.float32, kind="ExternalOutput")
    dk8_t = nc.dram_tensor("dk8s", [Bl * KL, E], dt.float8e4, kind="Internal")
    with tile.TileContext(nc) as tc:
        with ExitStack() as ctx:
            emit_attention(ctx, tc, k_t.ap(), q_t.ap(), w_t.ap(), m_t.ap(),
                           o_t.ap(), dk8_t.ap(), Bl, Q, KL, E, QB=QB)
    nc.compile()
    return nc


def kernel(k: np.ndarray, q: np.ndarray, W: np.ndarray, mask: np.ndarray,
           **run_kwargs) -> np.ndarray:
    assert k.shape == (B, K_LEN, EMB) and q.shape == (B, Q_LEN, EMB)
    assert W.shape == (EMB, EMB) and mask.shape == (B, Q_LEN, K_LEN)
    Bl = B // N_CORES
    nc = build_program(Bl, Q_LEN, K_LEN, EMB)
    in_maps = []
    for c in range(N_CORES):
        sl = slice(c * Bl, (c + 1) * Bl)
        in_maps.append({
            "k": np.ascontiguousarray(k[sl], dtype=np.float32),
            "q": np.ascontiguousarray(q[sl], dtype=np.float32),
            "W": np.ascontiguousarray(W, dtype=np.float32),
            "mask": np.ascontiguousarray(mask[sl], dtype=np.int32),
        })
    res = run_bass_kernel_spmd(nc, in_maps, core_ids=list(range(N_CORES)),
                               **run_kwargs)
    out = np.concatenate([r["out"] for r in res.results], axis=0)
    if run_kwargs.get("trace"):
        kernel.last_exec_time_ns = res.exec_time_ns
        kernel.last_result = res
    return out


kernel.last_exec_time_ns = None
kernel.last_result = None


# revision 21
# speedup vs baseline: 1.2653x; 1.2316x over previous
"""Trainium2 Bass kernel for masked bi-linear attention.

Computes, for full inputs
    k:    [B, KL, E] f32
    q:    [B, Q,  E] f32
    W:    [E, E]     f32
    mask: [B, Q, KL] i32 (0/1)
the reference
    qw    = q @ W                      [B, Q, E]
    s     = qw @ k^T                   [B, Q, KL]
    p     = softmax(s, axis=-1) * mask
    out   = p @ k                      [B, Q, E]

Sharding: data-parallel over B across 8 NeuronCores (2 batches/core),
W replicated. Each core runs the same Bass program on its B-slice.

Precision: q/W/k in fp16 for the qw and score matmuls (score abs error
must stay << 1 since softmax amplifies it); softmax in fp32 on ACT/DVE.

The PV matmul (p @ k) runs in fp8 e4m3 with DoubleRow perf mode: the
stationary holds two p^T chunks (2 fp8 weights/cell) and the moving
operand streams the two matching k8 chunks, halving the PE cycles of
the PV phase.  fp8 quantization of k alone would cost ~2.7e-2 rel
error (over the 2e-2 gate) because softmax rows are near one-hot, so a
top-1 sparse correction restores it: per query row, the largest masked
probability e_top and its key index are found with DVE max/max_index,
the fp8 residual row dk8[idx] = fp8(k - fp8(k)) is gathered from a
DRAM scratch by indirect DMA, and one extra fp8 matmul with stationary
diag(e_top) accumulates e_top * dk8[idx] into the same PSUM group.
Measured rel l2 err ~5e-3 (vs 3e-3 for the all-fp16 PV).

Engine layout (per q-tile): PE does only matmuls (scores, qw share,
deferred PV); all transposes (q^T, k^T, p^T) run on the DMA xbar
(dma_start_transpose, 16-bit, all on the sync ring -- concurrent
transposes on both HWDGE rings corrupt); the score psum is consumed
directly by the DVE row-max and the ACT exp (no psum->SBUF copy); the
fp16/fp8 input casts sit on ACT (q16/k16) and DVE (k8/dk8); max8 and
find_index8 for the top-1 correction on DVE; mask loads, dk8 spills,
residual gathers and output stores on the gpsimd (SWDGE) ring.
"""

import numpy as np

import concourse.bacc as bacc
import concourse.bass as bass
import concourse.mybir as mybir
import concourse.tile as tile
from concourse.bass_utils import run_bass_kernel_spmd
from concourse.masks import make_identity
from contextlib import ExitStack

dt = mybir.dt
AF = mybir.ActivationFunctionType
ALU = mybir.AluOpType
AX = mybir.AxisListType
PM = mybir.MatmulPerfMode

P = 128

N_CORES = 8
B, Q_LEN, K_LEN, EMB = 16, 2048, 2048, 1024


def emit_attention(ctx, tc, k_ap, q_ap, w_ap, mask_ap, out_ap, dk8_ap,
                   Bl, Q, KL, E, QB=512):
    """Emit the per-core attention program.

    k_ap [Bl, KL, E], q_ap [Bl, Q, E], w_ap [E, E], mask_ap [Bl, Q, KL],
    out_ap [Bl, Q, E], dk8_ap [Bl*KL, E] fp8 scratch (Internal DRAM).
    """
    nc = tc.nc
    f32, bf16, f16, i32 = dt.float32, dt.bfloat16, dt.float16, dt.int32
    f8, u32 = dt.float8e4, dt.uint32

    assert Q % QB == 0 and QB % P == 0 and KL % P == 0 and E % P == 0
    EC = E // P          # e (contraction for qw) chunks
    KC = KL // P         # k chunks
    FC = E // P          # f chunks (qw output tiles)
    nqb = Q // QB
    qt_per_b = QB // P
    KB = min(512, KL)    # score psum block (<= 1 bank)
    nkb = KL // KB
    EB = min(512, E)     # PV psum block
    neb = E // EB

    const = ctx.enter_context(tc.tile_pool(name="const", bufs=1))
    ident = const.tile([P, P], f32)
    make_identity(nc, ident[:])
    idh_t = const.tile([P, P], f16)
    nc.gpsimd.tensor_copy(idh_t[:], ident[:])   # 0/1 exact in fp16
    idh = idh_t[:]
    id8_t = const.tile([P, P], f8)
    nc.gpsimd.tensor_copy(id8_t[:], ident[:])   # 0/1 exact in fp8
    id8 = id8_t[:]
    GW = 4               # q/k transposes batched per psum bank

    big = ctx.enter_context(tc.tile_pool(name="big", bufs=1))
    qio = ctx.enter_context(tc.tile_pool(name="qio", bufs=5))
    q16p = ctx.enter_context(tc.tile_pool(name="q16p", bufs=3))
    mio = ctx.enter_context(tc.tile_pool(name="mio", bufs=2))
    ptp = ctx.enter_context(tc.tile_pool(name="ptp", bufs=4))
    work = ctx.enter_context(tc.tile_pool(name="work", bufs=2))
    small = ctx.enter_context(tc.tile_pool(name="small", bufs=3))
    psum = ctx.enter_context(tc.tile_pool(name="psum", bufs=4, space="PSUM"))
    psum_t = ctx.enter_context(tc.tile_pool(name="psum_t", bufs=2, space="PSUM"))
    psum_o = ctx.enter_context(tc.tile_pool(name="psum_o", bufs=1, space="PSUM"))

    # ---- W: loaded once per core as fp16; the DMA+cast emission happens
    # after the first q-block's loads so the kernel head starts on the q
    # pipeline instead of waiting for W
    wH = big.tile([P, EC * E], f16, tag="wH")

    def emit_w_load():
        for ec in range(EC):
            win = qio.tile([P, E], f32, tag="qin", name="win")
            nc.sync.dma_start(win[:], w_ap[ec * P:(ec + 1) * P, :])
            nc.scalar.copy(wH[:, ec * E:(ec + 1) * E], win[:])

    # deferred-PV state: [b, row0, spb, rz, k8t, e8q, idxa, (pT8, dkg)]
    pending = []

    def pv_prep(st):
        # p^T via the DMA xbar (SBUF->SBUF, bf16) on the sync queue and
        # the residual-row gather (gpsimd).  The fp8 cast of pT is NOT
        # emitted here: it would sit at the head of the DVE queue
        # waiting on the transpose and block the current tile's row-max
        # reduces -- pv_prep_cast() is emitted after the softmax chain
        spb, idxa = st[2], st[6]
        pT = ptp.tile([P, KC, P], bf16, tag="pT", name="pT", bufs=2)
        nc.sync.dma_start_transpose(pT[:], spb[:])
        dkg = mio.tile([P, E], f8, tag="dkg", name="dkg", bufs=2)
        nc.gpsimd.indirect_dma_start(
            out=dkg[:], out_offset=None,
            in_=dk8_ap[:],
            in_offset=bass.IndirectOffsetOnAxis(ap=idxa[:, :1], axis=0))
        st.append([pT, dkg, None])
        return st[-1]

    def pv_prep_cast(st):
        prep = st[7]
        pT = prep[0]
        pT8 = ptp.tile([P, KC, P], f8, tag="pT8", name="pT8", bufs=2)
        nc.vector.tensor_copy(pT8[:], pT[:])
        prep[2] = pT8

    def pv_mms(st, prep):
        b, row0, spb, rz, k8t, e8q = st[:6]
        pT, dkg, pT8 = prep
        # per-row correction scale diag(e_top) in fp8
        dg8 = small.tile([P, P], f8, tag="dg8", name="dg8", bufs=2)
        nc.vector.tensor_scalar_mul(dg8[:], id8, e8q[:, 0:1])
        po = [psum_o.tile([P, EB], f32, tag=f"po{eh}", name=f"po{eh}")
              for eh in range(neb)]
        for c2 in range(KC // 2):
            for eh in range(neb):
                nc.tensor.matmul(
                    po[eh][:], pT8[:, 2 * c2:2 * c2 + 2, :],
                    k8t[:, 2 * c2:2 * c2 + 2, eh * EB:(eh + 1) * EB],
                    start=(c2 == 0), stop=False,
                    perf_mode=PM.DoubleRow)
        for eh in range(neb):
            nc.tensor.matmul(
                po[eh][:], dg8[:], dkg[:, eh * EB:(eh + 1) * EB],
                start=False, stop=True)
        for eh in range(neb):
            ot = mio.tile([P, EB], f32, tag="ot", name="ot")
            nc.scalar.activation(ot[:], po[eh][:], AF.Copy, scale=rz[:])
            nc.gpsimd.dma_start(
                out_ap[b, row0: row0 + P, eh * EB:(eh + 1) * EB], ot[:])

    # ---- q loads: DMAs may be issued ahead (prefetched) of the cast
    # emission so they don't queue behind mask DMAs
    def emit_qin_dmas(b, qb, qts):
        tiles = []
        for qt in qts:
            qin = qio.tile([P, E], f32, tag="qin", name="qin")
            nc.sync.dma_start(
                qin[:], q_ap[b, qb * QB + qt * P: qb * QB + (qt + 1) * P, :])
            tiles.append(qin)
        return tiles

    def emit_q16(qins):
        outs = []
        for qin in qins:
            q16 = q16p.tile([P, E], f16, tag="q16", name="q16", bufs=5)
            nc.vector.tensor_copy(q16[:], qin[:])
            outs.append(q16)
        return outs

    def emit_block_qT(b, qb, pre16):
        qT = big.tile([P, EC, QB], f16, tag="qTh", name="qT")
        q16s = pre16 + emit_q16(
            emit_qin_dmas(b, qb, range(len(pre16), qt_per_b)))
        for qt in range(qt_per_b):
            q16 = q16s[qt]
            for eg in range(EC // GW):
                pt = psum_t.tile([P, GW * P], f16, tag="tp", name="pt")
                for j in range(GW):
                    ec = eg * GW + j
                    nc.tensor.transpose(
                        pt[:, j * P:(j + 1) * P],
                        q16[:, ec * P:(ec + 1) * P], idh)
                ptv = pt[:].rearrange("p (g c) -> p g c", g=GW)
                nc.scalar.copy(
                    qT[:, eg * GW:(eg + 1) * GW, qt * P:(qt + 1) * P], ptv)
        return qT

    def emit_block_qw(qT):
        qwT = big.tile([P, FC * QB], f16, tag="qwTh", name="qwT")
        for fc in range(FC):
            ps = psum.tile([P, QB], f32, tag="ps", name="ps")
            for ec in range(EC):
                nc.tensor.matmul(
                    ps[:], wH[:, ec * E + fc * P: ec * E + (fc + 1) * P],
                    qT[:, ec, :], start=(ec == 0), stop=(ec == EC - 1))
            nc.scalar.copy(qwT[:, fc * QB:(fc + 1) * QB], ps[:])
        return qwT

    def emit_k_phase(b):
        k8 = big.tile([P, KC, E], f8, tag="k8", name="k8", bufs=2)
        kTh = big.tile([P, EC, KL], f16, tag="kTh", name="kTh")

        def chunk(kc):
            kin = qio.tile([P, E], f32, tag="qin", name="kin")
            nc.sync.dma_start(kin[:], k_ap[b, kc * P:(kc + 1) * P, :])
            # fp8 rounding copy for the PV matmul rhs + residual row
            nc.vector.tensor_copy(k8[:, kc, :], kin[:])
            dk8 = q16p.tile([P, E], f8, tag="dk8", name="dk8", bufs=3)
            nc.vector.tensor_tensor(out=dk8[:], in0=kin[:],
                                    in1=k8[:, kc, :], op=ALU.subtract)
            nc.gpsimd.dma_start(
                dk8_ap[b * KL + kc * P: b * KL + (kc + 1) * P, :], dk8[:])
            k16 = q16p.tile([P, E], f16, tag="q16", name="k16", bufs=5)
            nc.vector.tensor_copy(k16[:], kin[:])
            for eg in range(EC // GW):
                pt = psum_t.tile([P, GW * P], f16, tag="tp", name="pt")
                for j in range(GW):
                    ec = eg * GW + j
                    nc.tensor.transpose(
                        pt[:, j * P:(j + 1) * P],
                        k16[:, ec * P:(ec + 1) * P], idh)
                ptv = pt[:].rearrange("p (g c) -> p g c", g=GW)
                nc.scalar.copy(
                    kTh[:, eg * GW:(eg + 1) * GW, kc * P:(kc + 1) * P], ptv)

        for kc in range(KC):
            chunk(kc)
        return k8, kTh

    for b in range(Bl):
        # first q-block prep runs before the K phase: its qw matmuls keep
        # the PE busy while the k DMA stream lands; k8 is
        # double-buffered so the previous batch's deferred PV drains
        # during the k load
        qT = emit_block_qT(b, 0, [])

        if b == 0:
            emit_w_load()
        qwT = emit_block_qw(qT)
        k8, kTh = emit_k_phase(b)

        qin_pre, q16_pre = [], []
        for qb in range(nqb):
            if qb > 0:
                qT = emit_block_qT(b, qb, q16_pre)
                q16_pre = []
                qwT = emit_block_qw(qT)

            for qt in range(qt_per_b):
                row0 = qb * QB + qt * P
                # mask prefetch on the gpsimd (SWDGE) ring, one [P, KB]
                # tile per score block, consumed after this tile's exp
                mts = []
                for kb in range(nkb):
                    mt = mio.tile([P, KB], i32, tag="mask", name="mt",
                                  bufs=6)
                    nc.gpsimd.dma_start(
                        mt[:], mask_ap[b, row0: row0 + P,
                                       kb * KB:(kb + 1) * KB])
                    mts.append(mt)

                # xbar-transpose + fp8-cast + residual-gather for the
                # newest deferred tile: its mask multiplies have
                # finished by the time the sync queue reaches this
                if pending:
                    pv_prep(pending[-1])

                sp = work.tile([P, KL], f32, tag="sp", name="sp", bufs=1)
                mx = small.tile([P, nkb], f32, tag="mx", name="mx")
                for kb in range(nkb):
                    ps_s = psum.tile([P, KB], f32, tag="ps", name="ps_s")
                    for fc in range(FC):
                        nc.tensor.matmul(
                            ps_s[:],
                            qwT[:, fc * QB + qt * P: fc * QB + (qt + 1) * P],
                            kTh[:, fc, kb * KB:(kb + 1) * KB],
                            start=(fc == 0), stop=(fc == FC - 1))
                    nc.scalar.copy(sp[:, kb * KB:(kb + 1) * KB], ps_s[:])
                    nc.vector.tensor_reduce(
                        mx[:, kb:kb + 1], sp[:, kb * KB:(kb + 1) * KB],
                        axis=AX.X, op=ALU.max)

                negm = small.tile([P, 1], f32, tag="negm", name="negm")
                nc.vector.tensor_reduce(negm[:], mx[:], axis=AX.X,
                                        op=ALU.max, negate=True)

                spb = work.tile([P, KL], bf16, tag="spb", name="spb")
                zs = small.tile([P, nkb], f32, tag="zs", name="zs")
                for kb in range(nkb):
                    blk = slice(kb * KB, (kb + 1) * KB)
                    nc.scalar.activation(spb[:, blk], sp[:, blk], AF.Exp,
                                         bias=negm[:],
                                         accum_out=zs[:, kb:kb + 1])
                    nc.vector.scalar_tensor_tensor(
                        out=spb[:, blk], in0=mts[kb][:], scalar=1.0,
                        in1=spb[:, blk], op0=ALU.mult, op1=ALU.mult)
                z = small.tile([P, 1], f32, tag="z", name="z")
                nc.vector.tensor_reduce(z[:], zs[:], axis=AX.X, op=ALU.add)
                rz = small.tile([P, 1], f32, tag="rz", name="rz")
                nc.vector.reciprocal(rz[:], z[:])

                # top-1 masked probability (value + key index) for the
                # fp8 residual correction
                m8 = small.tile([P, 8], bf16, tag="m8", name="m8")
                mi = small.tile([P, 8], u32, tag="mi", name="mi")
                nc.vector.max(m8[:], spb[:])
                nc.vector.max_index(mi[:], m8[:], spb[:])
                idxa = small.tile([P, 1], u32, tag="idxa", name="idxa")
                nc.vector.tensor_scalar_add(idxa[:], mi[:, 0:1], b * KL)
                # f32 copy of the top value; the write into the fp8
                # diag tile rounds it to the same e4m3 value the
                # DoubleRow stationary used
                e8q = small.tile([P, 1], f32, tag="e8q", name="e8q")
                nc.vector.tensor_copy(e8q[:], m8[:, 0:1])

                # the fp8 cast of the previous tile's pT, emitted at
                # the DVE queue tail so it never blocks this tile's
                # softmax chain
                if (pending and len(pending[-1]) > 7
                        and pending[-1][7][2] is None):
                    pv_prep_cast(pending[-1])

                # PV deferred by two tiles so the xbar transpose is
                # never on the critical path
                if len(pending) > 1:
                    st = pending.pop(0)
                    pv_mms(st, st[7])
                pending.append([b, row0, spb, rz, k8, e8q, idxa])

                # prefetch the next block's q rows (DMA a tile before
                # the fp16 cast, so neither the sync queue nor ACT
                # stalls): the block head then has no input dependency
                if qt == qt_per_b - 3 and qb + 1 < nqb:
                    qin_pre = emit_qin_dmas(b, qb + 1, range(4))
                if qt == qt_per_b - 2 and qin_pre:
                    q16_pre = emit_q16(qin_pre)
                    qin_pre = []

    for st in pending:
        if len(st) <= 7:
            pv_prep(st)
        if st[7][2] is None:
            pv_prep_cast(st)
    while pending:
        st = pending.pop(0)
        pv_mms(st, st[7])


def build_program(Bl, Q, KL, E, QB=512):
    nc = bacc.Bacc("TRN2", target_bir_lowering=False, debug=False)
    k_t = nc.dram_tensor("k", [Bl, KL, E], dt.float32, kind="ExternalInput")
    q_t = nc.dram_tensor("q", [Bl, Q, E], dt.float32, kind="ExternalInput")
    w_t = nc.dram_tensor("W", [E, E], dt.float32, kind="ExternalInput")
    m_t = nc.dram_tensor("mask", [Bl, Q, KL], dt.int32, kind="ExternalInput")
    o_t = nc.dram_tensor("out", [Bl, Q, E], dt.float32, kind="ExternalOutput")
    dk8_t = nc.dram_tensor("dk8s", [Bl * KL, E], dt.float8e4, kind="Internal")
    with tile.TileContext(nc) as tc:
        with ExitStack() as ctx:
            emit_attention(ctx, tc, k_t.ap(), q_t.ap(), w_t.ap(), m_t.ap(),
                           o_t.ap(), dk8_t.ap(), Bl, Q, KL, E, QB=QB)
    nc.compile()
    return nc


def kernel(k: np.ndarray, q: np.ndarray, W: np.ndarray, mask: np.ndarray,
           **run_kwargs) -> np.ndarray:
    assert k.shape == (B, K_LEN, EMB) and q.shape == (B, Q_LEN, EMB)
    assert W.shape == (EMB, EMB) and mask.shape == (B, Q_LEN, K_LEN)
    Bl = B // N_CORES
    nc = build_program(Bl, Q_LEN, K_LEN, EMB)
    in_maps = []
    for c in range(N_CORES):
        sl = slice(c * Bl, (c + 1) * Bl)
        in_maps.append({
            "k": np.ascontiguousarray(k[sl], dtype=np.float32),
            "q": np.ascontiguousarray(q[sl], dtype=np.float32),
            "W": np.ascontiguousarray(W, dtype=np.float32),
            "mask": np.ascontiguousarray(mask[sl], dtype=np.int32),
        })
    res = run_bass_kernel_spmd(nc, in_maps, core_ids=list(range(N_CORES)),
                               **run_kwargs)
    out = np.concatenate([r["out"] for r in res.results], axis=0)
    if run_kwargs.get("trace"):
        kernel.last_exec_time_ns = res.exec_time_ns
        kernel.last_result = res
    return out


kernel.last_exec_time_ns = None
kernel.last_result = None
